# revision 5
# baseline (speedup 1.0000x reference)
"""DIFF cross-attention kernel for 8 Trainium2 NeuronCores (v2).

Sharding: tensor-parallel over heads x data-parallel over batch.
Core r handles batch b = r//4 and head group g = r%4 (4 of 16 heads).

v2 structure (vs v1): the attention phase is ACT(exp)-bound, so everything
else is arranged to hide under it:
  - Prefix: two d-outer waves fully overlapped with the input DMA stream:
    wave1 = q-proj (mi=0) + k-proj (mi=0, kc=0) + v-proj kt0-3,
    wave2 = k-proj (mi=0, kc=1-3) + v-proj kt4-7.
  - Attention passes (hp=head-pair, qb=query-half): per key-tile ONE
    [128,1024] PSUM tile holds BOTH heads' scores (even head cols 0:512 via
    PE row-tile (0,0), odd head cols 512:1024 via row-tile (64,0) — the two
    K=64 matmuls run concurrently on the PE array), one exp covers both.
  - Branch 1's u accumulators evacuate PSUM->SBUF so PSUM fits
    2x sc(2 banks) + 2 u + 2 filler banks.
  - Remaining projections (q/k mi=1) and v-proj kt8-15 are emitted as PE
    filler work inside the attention loop (v with a 2-slot lead over its
    consumer).
  - One AllGather per (batch-group, query-half): qb0's collective+proj
    overlap the qb1 attention pass / AG.

Tail: RMSNorm's column scale commutes through the projection
(y = (W_nw x) * s + b), so norm_w is folded into the proj weights on
the host and the projection runs on raw gathered x with K=1024, each
core computing only its own 256 output rows -- no reduce needed.
Host returns out[b][t, c] = y_out[r][c', t].
"""

import numpy as np
import ml_dtypes

B = 2
NQ = 1024
NK = 2048
DIM = 1024
H = 16
HD = 64
NH = 4            # heads per core
G = 4             # cores per batch group
SCALE = 0.125
LAMBDA_INIT = 0.1
EPS = 1e-6
P = 128
DC = DIM // P     # 8 contraction chunks
KT = NK // P      # 16 key tiles
GROUPS = [[0, 1, 2, 3], [4, 5, 6, 7]]
AGR = 2 * P + 1   # rows per rank in the AllGather payload


def _build(stop_after="full", loop_n=0, pack=True):
    import concourse.bass as bass
    import concourse.tile as tile
    import concourse.mybir as mybir
    from concourse import bacc

    f32 = mybir.dt.float32
    bf16 = mybir.dt.bfloat16
    AF = mybir.ActivationFunctionType

    nc = bacc.Bacc("TRN2", target_bir_lowering=False, debug=False, num_devices=8)

    qT_d = nc.dram_tensor("qT", [DIM, NQ], bf16, kind="ExternalInput")
    kT_d = nc.dram_tensor("kT", [DIM, NK], bf16, kind="ExternalInput")
    wq1_d = nc.dram_tensor("wq1", [DIM, 256], bf16, kind="ExternalInput")
    wq2_d = nc.dram_tensor("wq2", [DIM, 256], bf16, kind="ExternalInput")
    wk1_d = nc.dram_tensor("wk1", [DIM, 256], bf16, kind="ExternalInput")
    wv1_d = nc.dram_tensor("wv1", [DIM, 256], bf16, kind="ExternalInput")
    wk2_d = nc.dram_tensor("wk2", [DIM, 256], bf16, kind="ExternalInput")
    # proj weights with norm_w folded in, rows pre-permuted to the
    # AllGather chunk order: chunk j=(r, mi) -> channels 256r+128mi..+128,
    # cols = this core's 256 output channels.
    wp_d = nc.dram_tensor("wp", [DIM, 256], bf16, kind="ExternalInput")
    pb_d = nc.dram_tensor("pb", [P, 2], f32, kind="ExternalInput")
    lamn_d = nc.dram_tensor("lamn", [1, NH], f32, kind="ExternalInput")
    y_out_d = nc.dram_tensor("y_out", [256, NQ], f32, kind="ExternalOutput")

    def _trace(tc):
        with (
            tc.tile_pool(name="res", bufs=1) as res,
            tc.tile_pool(name="dram", bufs=1, space="DRAM") as dram,
        ):
            # ---- resident tensors ----
            qT_sb = res.tile([P, DC, NQ], bf16)
            kt_sb = res.tile([P, DC, NK], bf16)
            wq1s = res.tile([P, DC, 256], bf16, name="wq1s")
            wq2s = res.tile([P, DC, 256], bf16, name="wq2s")
            wk1 = res.tile([P, DC, 256], bf16, name="wk1s")
            wk2 = res.tile([P, DC, 256], bf16, name="wk2s")
            wv1 = res.tile([P, DC, 256], bf16, name="wv1s")
            wp_sb = res.tile([P, DC, 256], bf16)
            pb = res.tile([P, 2], f32)
            lamn = res.tile([1, NH], f32)
            ones_f = res.tile([P, KT], bf16)
            ones4w = res.tile([4, P], bf16)
            lam64 = res.tile([64, NH], f32)
            eps_t = res.tile([P, 1], f32)

            q1T = res.tile([P, 2, NQ], bf16)
            q2T = res.tile([P, 2, NQ], bf16)
            k1T = res.tile([P, 2, NK], bf16)
            k2T = res.tile([P, 2, NK], bf16)
            v1e = res.tile([P, NH, KT, 128], bf16)
            xT = res.tile([P, 2, NQ], bf16)

            ag_in = [dram.tile([AGR, 512], bf16, name=f"agi{qb}")
                     for qb in range(2)]
            ag_out = [dram.tile([G * AGR, 512], bf16, name=f"ago{qb}")
                      for qb in range(2)]

            # ---- DMA stream, rough priority order. Config cost sits on
            # the issuing engine's sequencer (SP/DVE: HWDGE ~0.6us each,
            # transfers run parallel on the DMA queues; gpsimd: SWDGE ~1us
            # and engine-serialized) — so the critical q/k stream issues
            # from SP, weights from DVE, and tail-only loads from gpsimd.
            def rearr2(eng, t_, d_):
                for hh in range(2):
                    eng.dma_start(
                        t_[:, hh * 4:(hh + 1) * 4, :],
                        d_[hh * 512:(hh + 1) * 512, :].rearrange(
                            "(o p) n -> p o n", p=P))

            for d in range(DC):
                nc.sync.dma_start(kt_sb[:, d, :], kT_d[d * P:(d + 1) * P, :])
            for t_, d_ in ((wk1, wk1_d), (wk2, wk2_d), (wq1s, wq1_d),
                           (wq2s, wq2_d), (wv1, wv1_d)):
                rearr2(nc.scalar, t_, d_)
            for d in range(DC):
                nc.scalar.dma_start(qT_sb[:, d, :],
                                    qT_d[d * P:(d + 1) * P, :])
            rearr2(nc.gpsimd, wp_sb, wp_d)
            nc.gpsimd.dma_start(pb[:], pb_d[:])
            nc.gpsimd.dma_start(lamn[:], lamn_d[:])

            nc.vector.memset(ones_f[:], 1.0)
            nc.vector.memset(ones4w[:], 1.0)
            nc.vector.memset(eps_t[:], EPS)
            nc.gpsimd.memset(v1e[:, :, :, 0:64], 1.0)
            nc.gpsimd.partition_broadcast(lam64[:], lamn[:])

            def v_copy_out(pv, ktpair):
                """Copy a [P, 512] psum tile holding v-proj results for key
                tiles (2*ktpair, 2*ktpair+1) out to v1e."""
                for j in range(2):
                    kt = 2 * ktpair + j
                    for h in range(NH):
                        nc.vector.tensor_copy(
                            v1e[:, h, kt, 64:128],
                            pv[:, j * 256 + h * 64:j * 256 + h * 64 + 64])

            # ---- prefix: one d-outer wave (q-proj mi0 + k-proj mi0
            # kc0/kc1, 8 banks, gated on DMA arrival) ----
            with tc.tile_pool(name="ps_pre", bufs=8, space="PSUM") as ps_pre:
                psq = [ps_pre.tile([P, 512], f32, tag="acc", name=f"psq{i}")
                       for i in range(4)]
                psk = [ps_pre.tile([P, 512], f32, tag="acc", name=f"psk{i}")
                       for i in range(4)]
                # warm the PE HAM clock-gate during the DMA wait: dummy
                # matmuls into psq[0] (its real group later starts with
                # start=True, so junk contents are overwritten)
                for _ in range(48):
                    nc.tensor.matmul(psq[0][0:16, 0:16], ones_f[:, 0:16],
                                     ones_f[:, 0:16], start=True, stop=True)
                def k_d(d):
                    for i, ws in ((0, wk1), (1, wk2)):
                        for kc in range(2):
                            nc.tensor.matmul(
                                psk[i * 2 + kc][:], ws[:, d, 0:P],
                                kt_sb[:, d, kc * 512:(kc + 1) * 512],
                                start=(d == 0), stop=(d == DC - 1))

                def q_d(d):
                    for pj, ws in ((0, wq1s), (1, wq2s)):
                        for qc in range(2):
                            nc.tensor.matmul(
                                psq[pj * 2 + qc][:],
                                ws[:, d, 0:P],
                                qT_sb[:, d, qc * 512:(qc + 1) * 512],
                                start=(d == 0), stop=(d == DC - 1))

                # static emission order matched to DMA arrival (kT chunks
                # stream early on SP; qT lands later on the ACT queue) so
                # the in-order PE FIFO never parks an early k-chunk matmul
                # behind a q-chunk still in flight
                for step in ("k0", "k1", "k2", "k3", "q0", "k4", "q1",
                             "k5", "q2", "k6", "q3", "k7", "q4", "q5",
                             "q6", "q7"):
                    (k_d if step[0] == "k" else q_d)(int(step[1]))
                for pj, dst in ((0, q1T), (1, q2T)):
                    for qc in range(2):
                        nc.vector.tensor_copy(
                            dst[:, 0, qc * 512:(qc + 1) * 512],
                            psq[pj * 2 + qc][:])
                for i, dst in ((0, k1T), (1, k2T)):
                    for kc in range(2):
                        nc.vector.tensor_copy(
                            dst[:, 0, kc * 512:(kc + 1) * 512],
                            psk[i * 2 + kc][:])


            if stop_after == "prefix":
                nc.sync.dma_start(y_out_d[0:P, 0:512],
                                  q1T[:, 0, :].bitcast(f32))
                return
            if stop_after == "prefixk":
                nc.sync.dma_start(y_out_d[0:P, :],
                                  k1T[:, 0, :].bitcast(f32))
                nc.sync.dma_start(y_out_d[P:2 * P, 0:512].bitcast(bf16),
                                  v1e[:, 0, :, 0:64])
                nc.sync.dma_start(y_out_d[P:2 * P, 512:520].bitcast(bf16),
                                  v1e[:, 0, :, 64:65])
                return

            # ---- attention passes ----
            with (
                tc.tile_pool(name="ps_sc", bufs=2, space="PSUM") as ps_sc,
                tc.tile_pool(name="ps_u", bufs=2, space="PSUM") as ps_u,
                tc.tile_pool(name="ps_fil", bufs=2, space="PSUM") as ps_fil,
                tc.tile_pool(name="att", bufs=4) as att,
                tc.tile_pool(name="evac", bufs=4) as evac,
                tc.tile_pool(name="smal", bufs=4) as smal,
                tc.tile_pool(name="x2p", bufs=2) as x2p,
            ):
                def v_group(ktpair):
                    """v-proj for key tiles (2k, 2k+1) as a filler group."""
                    pv = ps_fil.tile([P, 512], f32, tag="fil",
                                     name=f"pv{ktpair}")
                    for j in range(2):
                        kt = 2 * ktpair + j
                        for d in range(DC):
                            nc.tensor.matmul(
                                pv[:, j * 256:(j + 1) * 256],
                                kt_sb[:, d, kt * P:(kt + 1) * P],
                                wv1[:, d, :],
                                start=(d == 0), stop=(d == DC - 1))
                    v_copy_out(pv[:], ktpair)

                def kc_group(ws, dst, kc):
                    """k-proj mi0 for key block kc as a filler group."""
                    pk = ps_fil.tile([P, 512], f32, tag="fil",
                                     name=f"pk{kc}")
                    for d in range(DC):
                        nc.tensor.matmul(
                            pk[:], ws[:, d, 0:P],
                            kt_sb[:, d, kc * 512:(kc + 1) * 512],
                            start=(d == 0), stop=(d == DC - 1))
                    nc.vector.tensor_copy(
                        dst[:, 0, kc * 512:(kc + 1) * 512], pk[:])

                # remaining projections emitted inside pass-1-br0, each
                # group just before the first slot that consumes it, so the
                # first exp fires ~15us earlier and ACT ramps while PE
                # drains these groups
                groups = [
                    (1, lambda: v_group(0)), (3, lambda: v_group(1)),
                    (5, lambda: v_group(2)), (7, lambda: v_group(3)),
                    (8, lambda: kc_group(wk1, k1T, 2)),
                    (9, lambda: v_group(4)),
                    (10, lambda: kc_group(wk2, k2T, 2)),
                    (11, lambda: v_group(5)),
                    (12, lambda: kc_group(wk1, k1T, 3)),
                    (13, lambda: v_group(6)),
                    (14, lambda: kc_group(wk2, k2T, 3)),
                    (15, lambda: v_group(7)),
                ]
                gq = list(groups)

                def emit_groups(slot):
                    while gq and gq[0][0] <= slot:
                        gq.pop(0)[1]()

                def filler_items():
                    """q-proj mi=1 then k-proj mi=1, one instruction per
                    item."""
                    for ws, dst in ((wq1s, q1T), (wq2s, q2T)):
                        for qc in range(2):
                            ft = ps_fil.tile([P, 512], f32, tag="fil",
                                             name=f"fq{qc}")
                            for d in range(DC):
                                yield lambda ft=ft, ws=ws, d=d, qc=qc: \
                                    nc.tensor.matmul(
                                        ft[:], ws[:, d, P:2 * P],
                                        qT_sb[:, d, qc * 512:(qc + 1) * 512],
                                        start=(d == 0), stop=(d == DC - 1))
                            yield lambda ft=ft, dst=dst, qc=qc: \
                                nc.vector.tensor_copy(
                                    dst[:, 1, qc * 512:(qc + 1) * 512], ft[:])
                    for ws, dst in ((wk1, k1T), (wk2, k2T)):
                        for kc in range(4):
                            ft = ps_fil.tile([P, 512], f32, tag="fil",
                                             name=f"fk{kc}")
                            for d in range(DC):
                                yield lambda ft=ft, ws=ws, d=d, kc=kc: \
                                    nc.tensor.matmul(
                                        ft[:], ws[:, d, P:2 * P],
                                        kt_sb[:, d, kc * 512:(kc + 1) * 512],
                                        start=(d == 0), stop=(d == DC - 1))
                            yield lambda ft=ft, dst=dst, kc=kc: \
                                nc.vector.tensor_copy(
                                    dst[:, 1, kc * 512:(kc + 1) * 512], ft[:])

                filler = filler_items()
                fill_done = False

                def emit_fill(k):
                    nonlocal fill_done
                    if fill_done:
                        return
                    for _ in range(k):
                        it = next(filler, None)
                        if it is None:
                            fill_done = True
                            return
                        it()

                for hp in range(2):
                    he, ho = 2 * hp, 2 * hp + 1
                    for qb in range(2):
                        qs = slice(qb * 512, (qb + 1) * 512)
                        u_prev = None
                        for br, (ktp, qtp) in enumerate(
                                ((k1T, q1T), (k2T, q2T))):
                            u_e = ps_u.tile([P, 512], f32, tag="u")
                            u_o = ps_u.tile([P, 512], f32, tag="u")
                            # software pipeline: the u-matmuls consuming
                            # slot k's exp are emitted during slot k+1, so
                            # the PE never sits in-FIFO behind the ACT exp
                            pend = None

                            def flush_u():
                                nonlocal pend
                                if pend is None:
                                    return
                                e_p, kt_p = pend
                                nc.tensor.matmul(
                                    u_e[:], v1e[:, he, kt_p, :],
                                    e_p[:, 0:512],
                                    start=(kt_p == 0), stop=(kt_p == KT - 1))
                                nc.tensor.matmul(
                                    u_o[:], v1e[:, ho, kt_p, :],
                                    e_p[:, 512:1024],
                                    start=(kt_p == 0), stop=(kt_p == KT - 1))
                                pend = None

                            for kt in range(KT):
                                if hp == 0 and qb == 0 and br == 0:
                                    emit_groups(kt)
                                sc = ps_sc.tile([P, 1024], f32, tag="sc")
                                nc.tensor.matmul(
                                    sc[:, 0:512],
                                    ktp[0:64, hp, kt * P:(kt + 1) * P],
                                    qtp[0:64, hp, qs],
                                    start=True, stop=True,
                                    tile_position=(0, 0) if pack else None)
                                nc.tensor.matmul(
                                    sc[:, 512:1024],
                                    ktp[64:128, hp, kt * P:(kt + 1) * P],
                                    qtp[64:128, hp, qs],
                                    start=True, stop=True,
                                    tile_position=(64, 0) if pack else None)
                                e_t = att.tile([P, 1024], bf16, tag="e")
                                nc.scalar.activation(e_t[:], sc[:],
                                                     AF.Exp, scale=SCALE)
                                flush_u()
                                pend = (e_t, kt)
                                if not gq:
                                    emit_fill(3)
                            flush_u()
                            if br == 0:
                                # reduce branch 1 to t1 = u1/l1 right here:
                                # frees the u banks as fast as an evacuation
                                # copy would, and takes the rr1/t1 ops off
                                # the post-last-exp critical tail
                                tps = []
                                for u1 in (u_e, u_o):
                                    rr1b = smal.tile([64, 512], f32,
                                                     tag="rrb")
                                    nc.vector.reciprocal_approx_fast(
                                        rr1b[:], u1[0:64, :])
                                    t1 = evac.tile([64, 512], f32, tag="u1")
                                    nc.vector.tensor_mul(
                                        t1[:], u1[64:128, :], rr1b[:])
                                    tps.append(t1)
                                u_prev = tuple(tps)
                            else:
                                for head, po, t1, u2 in (
                                        (he, 0, u_prev[0], u_e),
                                        (ho, 64, u_prev[1], u_o)):
                                    rr2b = smal.tile([64, 512], f32,
                                                     tag="rrb")
                                    nc.vector.reciprocal_approx_fast(
                                        rr2b[:], u2[0:64, :])
                                    nc.vector.tensor_scalar_mul(
                                        rr2b[:], rr2b[:],
                                        lam64[:, head:head + 1])
                                    t2 = smal.tile([64, 512], f32, tag="tt")
                                    nc.vector.tensor_mul(
                                        t2[:], u2[64:128, :], rr2b[:])
                                    nc.vector.tensor_add(
                                        xT[po:po + 64, hp, qs], t1[:], t2[:])
                        if hp == 1 and stop_after == "full":
                            # ss + AllGather for this query half
                            for mi in range(2):
                                nc.sync.dma_start(
                                    ag_in[qb][mi * P:(mi + 1) * P, :],
                                    xT[:, mi, qs])
                            ssp = ps_fil.tile([P, 512], f32, tag="fil",
                                              name=f"ss{qb}")
                            for mi in range(2):
                                x2c = x2p.tile([P, 512], bf16, tag="x2")
                                nc.vector.tensor_mul(
                                    x2c[:], xT[:, mi, qs], xT[:, mi, qs])
                                nc.tensor.matmul(
                                    ssp[0:1, :], ones_f[:, 0:1], x2c[:],
                                    start=(mi == 0), stop=(mi == 1))
                            ss_sb = x2p.tile([1, 512], bf16, tag="sssb")
                            nc.vector.tensor_copy(ss_sb[:], ssp[0:1, :])
                            nc.sync.dma_start(
                                ag_in[qb][2 * P:2 * P + 1, :], ss_sb[:])
                            nc.gpsimd.collective_compute(
                                "AllGather",
                                mybir.AluOpType.bypass,
                                replica_groups=GROUPS,
                                ins=[ag_in[qb].opt()],
                                outs=[ag_out[qb].opt()],
                            )

            if stop_after == "attn":
                nc.sync.dma_start(y_out_d[0:P, 0:512],
                                  xT[:, 0, :].bitcast(f32))
                return

            # ---- tail: gather, RMS scale, out-projection per query half ----
            with (
                tc.tile_pool(name="post", bufs=1) as post,
                tc.tile_pool(name="postr", bufs=2) as postr,
                tc.tile_pool(name="ps_yp", bufs=2, space="PSUM") as ps_yp,
                tc.tile_pool(name="ps_s", bufs=2, space="PSUM") as ps_s,
            ):
                for qb in range(2):
                    if qb == 1:
                        # keep the PE HAM clock-gate open while AG1 flies
                        warm = ps_yp.tile([P, 512], f32, tag="yp")
                        for _ in range(50):
                            nc.tensor.matmul(
                                warm[0:16, 0:16], ones_f[:, 0:16],
                                ones_f[:, 0:16], start=True, stop=True)
                    qs = slice(qb * 512, (qb + 1) * 512)
                    xall = post.tile([P, DC, 512], bf16, name=f"xall{qb}")
                    ss4 = post.tile([4, 512], bf16, name=f"ss4{qb}")
                    for r in range(G):
                        nc.sync.dma_start(
                            ss4[r:r + 1, :],
                            ag_out[qb][r * AGR + 2 * P:r * AGR + 2 * P + 1, :])

                    s_ps = ps_s.tile([P, 512], f32, tag="sps")
                    nc.tensor.matmul(s_ps[:], ones4w[:], ss4[:],
                                     start=True, stop=True)
                    # s = 1/sqrt(ms+eps) = exp(-0.5*ln(ms+eps)): Ln and
                    # Exp share an ACT table set, Sqrt does not — this
                    # avoids a ~2.7us table switch on the critical tail.
                    s_b = post.tile([P, 512], f32, name=f"sb{qb}")
                    nc.scalar.activation(s_b[:], s_ps[:], AF.Ln,
                                         bias=eps_t[:],
                                         scale=1.0 / DIM)
                    nc.scalar.activation(s_b[:], s_b[:], AF.Exp,
                                         scale=-0.5)

                    for r in range(G):
                        nc.sync.dma_start(
                            xall[:, r * 2:r * 2 + 2, :],
                            ag_out[qb][r * AGR:r * AGR + 2 * P, :].rearrange(
                                "(o p) n -> p o n", p=P))
                    for m in range(2):
                        yp = ps_yp.tile([P, 512], f32, tag="yp")
                        for j in range(DC):
                            nc.tensor.matmul(
                                yp[:],
                                wp_sb[:, j, m * P:(m + 1) * P],
                                xall[:, j, :],
                                start=(j == 0), stop=(j == DC - 1))
                        y_sb = postr.tile([P, 512], f32, tag="ysb")
                        nc.vector.tensor_mul(y_sb[:], yp[:], s_b[:])
                        nc.vector.tensor_scalar_add(y_sb[:], y_sb[:],
                                                    pb[:, m:m + 1])
                        nc.sync.dma_start(
                            y_out_d[m * P:(m + 1) * P, qs], y_sb[:])

    with tile.TileContext(nc) as tc:
        if loop_n:
            with tc.For_i(0, loop_n, 1):
                _trace(tc)
        else:
            _trace(tc)
    nc.compile()
    return nc


_CACHE = {}


def _get_nc():
    if "nc" not in _CACHE:
        _CACHE["nc"] = _build()
    return _CACHE["nc"]


def _shard_inputs(inputs):
    bf = ml_dtypes.bfloat16
    q = np.asarray(inputs["query"], np.float32)
    k = np.asarray(inputs["key"], np.float32)
    q1_w = np.asarray(inputs["q1_w"], np.float32)
    q2_w = np.asarray(inputs["q2_w"], np.float32)
    kv1_w = np.asarray(inputs["kv1_w"], np.float32)
    kv2_w = np.asarray(inputs["kv2_w"], np.float32)
    proj_w = np.asarray(inputs["proj_w"], np.float32)
    proj_b = np.asarray(inputs["proj_b"], np.float32)
    norm_w = np.asarray(inputs["norm_w"], np.float32)
    lam1 = np.asarray(inputs["lambda_1"], np.float32).reshape(H)
    lam2 = np.asarray(inputs["lambda_2"], np.float32).reshape(H)
    lam_full = lam1 - lam2 + LAMBDA_INIT

    # proj with norm folded in: y[o, q] = sum_c wpnw[c, o] x[c, q]
    wpnw = (proj_w * norm_w[None, :]).T  # [c, o]

    def c(x):
        return np.ascontiguousarray(x).astype(bf)

    in_maps = []
    for r in range(8):
        b, g = r // G, r % G
        rows = slice(g * 256, (g + 1) * 256)
        vrows = slice(DIM + g * 256, DIM + (g + 1) * 256)
        # AllGather chunk order: j = 2*rank + mi -> channels 256*rank+128*mi
        wp_perm = np.concatenate(
            [wpnw[rr * 256 + mi * 128: rr * 256 + (mi + 1) * 128, rows]
             for rr in range(G) for mi in range(2)], axis=0)
        in_maps.append({
            "qT": c(q[b].T),
            "kT": c(k[b].T),
            "wq1": c(q1_w[rows].T),
            "wq2": c(q2_w[rows].T),
            "wk1": c(kv1_w[rows].T),
            "wv1": c(kv1_w[vrows].T),
            "wk2": c(kv2_w[rows].T),
            "wp": c(wp_perm),
            "pb": np.ascontiguousarray(
                proj_b[rows].reshape(2, P).T).astype(np.float32),
            "lamn": np.ascontiguousarray(
                -lam_full[g * NH:(g + 1) * NH].reshape(1, NH)
            ).astype(np.float32),
        })
    return in_maps


def kernel(**inputs):
    from concourse.bass_utils import run_bass_kernel_spmd

    nc = _get_nc()
    in_maps = _shard_inputs(inputs)
    res = run_bass_kernel_spmd(nc, in_maps, core_ids=list(range(8)))
    out = np.empty((B, NQ, DIM), np.float32)
    for r in range(8):
        b, g = r // G, r % G
        out[b, :, g * 256:(g + 1) * 256] = res.results[r]["y_out"].T
    return out


# revision 6
# speedup vs baseline: 1.1115x; 1.1115x over previous
"""DIFF cross-attention kernel for 8 Trainium2 NeuronCores (v2).

Sharding: tensor-parallel over heads x data-parallel over batch.
Core r handles batch b = r//4 and head group g = r%4 (4 of 16 heads).

v2 structure (vs v1): the attention phase is ACT(exp)-bound, so everything
else is arranged to hide under it:
  - Prefix: two d-outer waves fully overlapped with the input DMA stream:
    wave1 = q-proj (mi=0) + k-proj (mi=0, kc=0) + v-proj kt0-3,
    wave2 = k-proj (mi=0, kc=1-3) + v-proj kt4-7.
  - Attention passes (hp=head-pair, qb=query-half): per key-tile ONE
    [128,1024] PSUM tile holds BOTH heads' scores (even head cols 0:512 via
    PE row-tile (0,0), odd head cols 512:1024 via row-tile (64,0) — the two
    K=64 matmuls run concurrently on the PE array), one exp covers both.
  - Branch 1's u accumulators evacuate PSUM->SBUF so PSUM fits
    2x sc(2 banks) + 2 u + 2 filler banks.
  - Remaining projections (q/k mi=1) and v-proj kt8-15 are emitted as PE
    filler work inside the attention loop (v with a 2-slot lead over its
    consumer).
  - One AllGather per (batch-group, query-half): qb0's collective+proj
    overlap the qb1 attention pass / AG.

Tail: RMSNorm's column scale commutes through the projection
(y = (W_nw x) * s + b), so norm_w is folded into the proj weights on
the host and the projection runs on raw gathered x with K=1024, each
core computing only its own 256 output rows -- no reduce needed.
Host returns out[b][t, c] = y_out[r][c', t].
"""

import numpy as np
import ml_dtypes

B = 2
NQ = 1024
NK = 2048
DIM = 1024
H = 16
HD = 64
NH = 4            # heads per core
G = 4             # cores per batch group
SCALE = 0.125
LAMBDA_INIT = 0.1
EPS = 1e-6
P = 128
DC = DIM // P     # 8 contraction chunks
KT = NK // P      # 16 key tiles
GROUPS = [[0, 1, 2, 3], [4, 5, 6, 7]]
AGR = 2 * P + 1   # rows per rank in the AllGather payload


def _build(stop_after="full", loop_n=0, pack=True):
    import concourse.bass as bass
    import concourse.tile as tile
    import concourse.mybir as mybir
    from concourse import bacc

    f32 = mybir.dt.float32
    bf16 = mybir.dt.bfloat16
    AF = mybir.ActivationFunctionType

    nc = bacc.Bacc("TRN2", target_bir_lowering=False, debug=False, num_devices=8)

    qT_d = nc.dram_tensor("qT", [DIM, NQ], bf16, kind="ExternalInput")
    kT_d = nc.dram_tensor("kT", [DIM, NK], bf16, kind="ExternalInput")
    wq1_d = nc.dram_tensor("wq1", [DIM, 256], bf16, kind="ExternalInput")
    wq2_d = nc.dram_tensor("wq2", [DIM, 256], bf16, kind="ExternalInput")
    wk1_d = nc.dram_tensor("wk1", [DIM, 256], bf16, kind="ExternalInput")
    wv1_d = nc.dram_tensor("wv1", [DIM, 256], bf16, kind="ExternalInput")
    wk2_d = nc.dram_tensor("wk2", [DIM, 256], bf16, kind="ExternalInput")
    # proj weights with norm_w folded in, rows pre-permuted to the
    # AllGather chunk order: chunk j=(r, mi) -> channels 256r+128mi..+128,
    # cols = this core's 256 output channels.
    wp_d = nc.dram_tensor("wp", [DIM, 256], bf16, kind="ExternalInput")
    pb_d = nc.dram_tensor("pb", [P, 2], f32, kind="ExternalInput")
    lamn_d = nc.dram_tensor("lamn", [1, NH], f32, kind="ExternalInput")
    y_out_d = nc.dram_tensor("y_out", [256, NQ], f32, kind="ExternalOutput")

    def _trace(tc):
        with (
            tc.tile_pool(name="res", bufs=1) as res,
            tc.tile_pool(name="dram", bufs=1, space="DRAM") as dram,
        ):
            # ---- resident tensors ----
            qT_sb = res.tile([P, DC, NQ], bf16)
            kt_sb = res.tile([P, DC, NK], bf16)
            wq1s = res.tile([P, DC, 256], bf16, name="wq1s")
            wq2s = res.tile([P, DC, 256], bf16, name="wq2s")
            wk1 = res.tile([P, DC, 256], bf16, name="wk1s")
            wk2 = res.tile([P, DC, 256], bf16, name="wk2s")
            wv1 = res.tile([P, DC, 256], bf16, name="wv1s")
            wp_sb = res.tile([P, DC, 256], bf16)
            pb = res.tile([P, 2], f32)
            lamn = res.tile([1, NH], f32)
            ones_f = res.tile([P, KT], bf16)
            ones4w = res.tile([4, P], bf16)
            lam64 = res.tile([64, NH], f32)
            eps_t = res.tile([P, 1], f32)

            q1T = res.tile([P, 2, NQ], bf16)
            q2T = res.tile([P, 2, NQ], bf16)
            k1T = res.tile([P, 2, NK], bf16)
            k2T = res.tile([P, 2, NK], bf16)
            v1e = res.tile([P, NH, KT, 128], bf16)
            xT = res.tile([P, 2, NQ], bf16)

            ag_in = [dram.tile([AGR, 512], bf16, name=f"agi{qb}")
                     for qb in range(2)]
            ag_out = [dram.tile([G * AGR, 512], bf16, name=f"ago{qb}")
                      for qb in range(2)]

            # ---- DMA stream, rough priority order. Config cost sits on
            # the issuing engine's sequencer (SP/DVE: HWDGE ~0.6us each,
            # transfers run parallel on the DMA queues; gpsimd: SWDGE ~1us
            # and engine-serialized) — so the critical q/k stream issues
            # from SP, weights from DVE, and tail-only loads from gpsimd.
            def rearr2(eng, t_, d_):
                for hh in range(2):
                    eng.dma_start(
                        t_[:, hh * 4:(hh + 1) * 4, :],
                        d_[hh * 512:(hh + 1) * 512, :].rearrange(
                            "(o p) n -> p o n", p=P))

            for d in range(DC):
                nc.sync.dma_start(kt_sb[:, d, :], kT_d[d * P:(d + 1) * P, :])
            for t_, d_ in ((wk1, wk1_d), (wk2, wk2_d), (wq1s, wq1_d),
                           (wq2s, wq2_d), (wv1, wv1_d)):
                rearr2(nc.scalar, t_, d_)
            for d in range(DC):
                nc.scalar.dma_start(qT_sb[:, d, :],
                                    qT_d[d * P:(d + 1) * P, :])
            rearr2(nc.gpsimd, wp_sb, wp_d)
            nc.gpsimd.dma_start(pb[:], pb_d[:])
            nc.gpsimd.dma_start(lamn[:], lamn_d[:])

            nc.vector.memset(ones_f[:], 1.0)
            nc.vector.memset(ones4w[:], 1.0)
            nc.vector.memset(eps_t[:], EPS)
            nc.gpsimd.memset(v1e[:, :, :, 0:64], 1.0)
            nc.gpsimd.partition_broadcast(lam64[:], lamn[:])

            def v_copy_out(pv, ktpair):
                """Copy a [P, 512] psum tile holding v-proj results for key
                tiles (2*ktpair, 2*ktpair+1) out to v1e."""
                for j in range(2):
                    kt = 2 * ktpair + j
                    for h in range(NH):
                        nc.vector.tensor_copy(
                            v1e[:, h, kt, 64:128],
                            pv[:, j * 256 + h * 64:j * 256 + h * 64 + 64])

            # ---- prefix: one d-outer wave (q-proj mi0 + k-proj mi0
            # kc0/kc1, 8 banks, gated on DMA arrival) ----
            with tc.tile_pool(name="ps_pre", bufs=8, space="PSUM") as ps_pre:
                psq = [ps_pre.tile([P, 512], f32, tag="acc", name=f"psq{i}")
                       for i in range(4)]
                psk = [ps_pre.tile([P, 512], f32, tag="acc", name=f"psk{i}")
                       for i in range(4)]
                # warm the PE HAM clock-gate during the DMA wait: dummy
                # matmuls into psq[0] (its real group later starts with
                # start=True, so junk contents are overwritten)
                for _ in range(48):
                    nc.tensor.matmul(psq[0][0:16, 0:16], ones_f[:, 0:16],
                                     ones_f[:, 0:16], start=True, stop=True)
                def k_d(d):
                    for i, ws in ((0, wk1), (1, wk2)):
                        for kc in range(2):
                            nc.tensor.matmul(
                                psk[i * 2 + kc][:], ws[:, d, 0:P],
                                kt_sb[:, d, kc * 512:(kc + 1) * 512],
                                start=(d == 0), stop=(d == DC - 1))

                def q_d(d):
                    for pj, ws in ((0, wq1s), (1, wq2s)):
                        for qc in range(2):
                            nc.tensor.matmul(
                                psq[pj * 2 + qc][:],
                                ws[:, d, 0:P],
                                qT_sb[:, d, qc * 512:(qc + 1) * 512],
                                start=(d == 0), stop=(d == DC - 1))

                # static emission order matched to DMA arrival (kT chunks
                # stream early on SP; qT lands later on the ACT queue) so
                # the in-order PE FIFO never parks an early k-chunk matmul
                # behind a q-chunk still in flight
                for step in ("k0", "k1", "k2", "k3", "q0", "k4", "q1",
                             "k5", "q2", "k6", "q3", "k7", "q4", "q5",
                             "q6", "q7"):
                    (k_d if step[0] == "k" else q_d)(int(step[1]))
                for pj, dst in ((0, q1T), (1, q2T)):
                    for qc in range(2):
                        nc.vector.tensor_copy(
                            dst[:, 0, qc * 512:(qc + 1) * 512],
                            psq[pj * 2 + qc][:])
                for i, dst in ((0, k1T), (1, k2T)):
                    for kc in range(2):
                        nc.vector.tensor_copy(
                            dst[:, 0, kc * 512:(kc + 1) * 512],
                            psk[i * 2 + kc][:])


            if stop_after == "prefix":
                nc.sync.dma_start(y_out_d[0:P, 0:512],
                                  q1T[:, 0, :].bitcast(f32))
                return
            if stop_after == "prefixk":
                nc.sync.dma_start(y_out_d[0:P, :],
                                  k1T[:, 0, :].bitcast(f32))
                nc.sync.dma_start(y_out_d[P:2 * P, 0:512].bitcast(bf16),
                                  v1e[:, 0, :, 0:64])
                nc.sync.dma_start(y_out_d[P:2 * P, 512:520].bitcast(bf16),
                                  v1e[:, 0, :, 64:65])
                return

            # ---- attention passes ----
            with (
                tc.tile_pool(name="ps_sc", bufs=2, space="PSUM") as ps_sc,
                tc.tile_pool(name="ps_u", bufs=2, space="PSUM") as ps_u,
                tc.tile_pool(name="ps_fil", bufs=2, space="PSUM") as ps_fil,
                tc.tile_pool(name="att", bufs=4) as att,
                tc.tile_pool(name="evac", bufs=4) as evac,
                tc.tile_pool(name="smal", bufs=4) as smal,
                tc.tile_pool(name="x2p", bufs=2) as x2p,
            ):
                def v_group(ktpair):
                    """v-proj for key tiles (2k, 2k+1) as a filler group."""
                    pv = ps_fil.tile([P, 512], f32, tag="fil",
                                     name=f"pv{ktpair}")
                    for j in range(2):
                        kt = 2 * ktpair + j
                        for d in range(DC):
                            nc.tensor.matmul(
                                pv[:, j * 256:(j + 1) * 256],
                                kt_sb[:, d, kt * P:(kt + 1) * P],
                                wv1[:, d, :],
                                start=(d == 0), stop=(d == DC - 1))
                    v_copy_out(pv[:], ktpair)

                def kc_group(ws, dst, kc):
                    """k-proj mi0 for key block kc as a filler group."""
                    pk = ps_fil.tile([P, 512], f32, tag="fil",
                                     name=f"pk{kc}")
                    for d in range(DC):
                        nc.tensor.matmul(
                            pk[:], ws[:, d, 0:P],
                            kt_sb[:, d, kc * 512:(kc + 1) * 512],
                            start=(d == 0), stop=(d == DC - 1))
                    nc.vector.tensor_copy(
                        dst[:, 0, kc * 512:(kc + 1) * 512], pk[:])

                # remaining projections emitted inside pass-1-br0, each
                # group just before the first slot that consumes it, so the
                # first exp fires ~15us earlier and ACT ramps while PE
                # drains these groups
                groups = [
                    (1, lambda: v_group(0)), (3, lambda: v_group(1)),
                    (5, lambda: v_group(2)), (7, lambda: v_group(3)),
                    (8, lambda: kc_group(wk1, k1T, 2)),
                    (9, lambda: v_group(4)),
                    (10, lambda: kc_group(wk2, k2T, 2)),
                    (11, lambda: v_group(5)),
                    (12, lambda: kc_group(wk1, k1T, 3)),
                    (13, lambda: v_group(6)),
                    (14, lambda: kc_group(wk2, k2T, 3)),
                    (15, lambda: v_group(7)),
                ]
                gq = list(groups)

                def emit_groups(slot):
                    while gq and gq[0][0] <= slot:
                        gq.pop(0)[1]()

                def filler_items():
                    """q-proj mi=1 then k-proj mi=1, one instruction per
                    item."""
                    for ws, dst in ((wq1s, q1T), (wq2s, q2T)):
                        for qc in range(2):
                            ft = ps_fil.tile([P, 512], f32, tag="fil",
                                             name=f"fq{qc}")
                            for d in range(DC):
                                yield lambda ft=ft, ws=ws, d=d, qc=qc: \
                                    nc.tensor.matmul(
                                        ft[:], ws[:, d, P:2 * P],
                                        qT_sb[:, d, qc * 512:(qc + 1) * 512],
                                        start=(d == 0), stop=(d == DC - 1))
                            yield lambda ft=ft, dst=dst, qc=qc: \
                                nc.vector.tensor_copy(
                                    dst[:, 1, qc * 512:(qc + 1) * 512], ft[:])
                    for ws, dst in ((wk1, k1T), (wk2, k2T)):
                        for kc in range(4):
                            ft = ps_fil.tile([P, 512], f32, tag="fil",
                                             name=f"fk{kc}")
                            for d in range(DC):
                                yield lambda ft=ft, ws=ws, d=d, kc=kc: \
                                    nc.tensor.matmul(
                                        ft[:], ws[:, d, P:2 * P],
                                        kt_sb[:, d, kc * 512:(kc + 1) * 512],
                                        start=(d == 0), stop=(d == DC - 1))
                            yield lambda ft=ft, dst=dst, kc=kc: \
                                nc.vector.tensor_copy(
                                    dst[:, 1, kc * 512:(kc + 1) * 512], ft[:])

                filler = filler_items()
                fill_done = False

                def emit_fill(k):
                    nonlocal fill_done
                    if fill_done:
                        return
                    for _ in range(k):
                        it = next(filler, None)
                        if it is None:
                            fill_done = True
                            return
                        it()

                for hp in range(2):
                    he, ho = 2 * hp, 2 * hp + 1
                    for qb in range(2):
                        qs = slice(qb * 512, (qb + 1) * 512)
                        u_prev = None
                        for br, (ktp, qtp) in enumerate(
                                ((k1T, q1T), (k2T, q2T))):
                            u_e = ps_u.tile([P, 512], f32, tag="u")
                            u_o = ps_u.tile([P, 512], f32, tag="u")
                            # software pipeline: the u-matmuls consuming
                            # slot k's exp are emitted during slot k+1, so
                            # the PE never sits in-FIFO behind the ACT exp
                            pend = None

                            def flush_u():
                                nonlocal pend
                                if pend is None:
                                    return
                                e_p, kt_p = pend
                                nc.tensor.matmul(
                                    u_e[:], v1e[:, he, kt_p, :],
                                    e_p[:, 0:512],
                                    start=(kt_p == 0), stop=(kt_p == KT - 1))
                                nc.tensor.matmul(
                                    u_o[:], v1e[:, ho, kt_p, :],
                                    e_p[:, 512:1024],
                                    start=(kt_p == 0), stop=(kt_p == KT - 1))
                                pend = None

                            for kt in range(KT):
                                if hp == 0 and qb == 0 and br == 0:
                                    emit_groups(kt)
                                sc = ps_sc.tile([P, 1024], f32, tag="sc")
                                nc.tensor.matmul(
                                    sc[:, 0:512],
                                    ktp[0:64, hp, kt * P:(kt + 1) * P],
                                    qtp[0:64, hp, qs],
                                    start=True, stop=True,
                                    tile_position=(0, 0) if pack else None)
                                nc.tensor.matmul(
                                    sc[:, 512:1024],
                                    ktp[64:128, hp, kt * P:(kt + 1) * P],
                                    qtp[64:128, hp, qs],
                                    start=True, stop=True,
                                    tile_position=(64, 0) if pack else None)
                                e_t = att.tile([P, 1024], bf16, tag="e")
                                nc.scalar.activation(e_t[:], sc[:],
                                                     AF.Exp, scale=SCALE)
                                flush_u()
                                pend = (e_t, kt)
                                if not gq:
                                    emit_fill(3)
                            flush_u()
                            if br == 0:
                                # reduce branch 1 to t1 = u1/l1 right here:
                                # frees the u banks as fast as an evacuation
                                # copy would, and takes the rr1/t1 ops off
                                # the post-last-exp critical tail
                                tps = []
                                for u1 in (u_e, u_o):
                                    rr1b = smal.tile([64, 512], f32,
                                                     tag="rrb")
                                    nc.vector.reciprocal_approx_fast(
                                        rr1b[:], u1[0:64, :])
                                    t1 = evac.tile([64, 512], f32, tag="u1")
                                    nc.vector.tensor_mul(
                                        t1[:], u1[64:128, :], rr1b[:])
                                    tps.append(t1)
                                u_prev = tuple(tps)
                            else:
                                for head, po, t1, u2 in (
                                        (he, 0, u_prev[0], u_e),
                                        (ho, 64, u_prev[1], u_o)):
                                    rr2b = smal.tile([64, 512], f32,
                                                     tag="rrb")
                                    nc.vector.reciprocal_approx_fast(
                                        rr2b[:], u2[0:64, :])
                                    nc.vector.tensor_scalar_mul(
                                        rr2b[:], rr2b[:],
                                        lam64[:, head:head + 1])
                                    t2 = smal.tile([64, 512], f32, tag="tt")
                                    nc.vector.tensor_mul(
                                        t2[:], u2[64:128, :], rr2b[:])
                                    nc.vector.tensor_add(
                                        xT[po:po + 64, hp, qs], t1[:], t2[:])
                        if hp == 1 and stop_after == "full":
                            # ss + AllGather for this query half
                            for mi in range(2):
                                nc.sync.dma_start(
                                    ag_in[qb][mi * P:(mi + 1) * P, :],
                                    xT[:, mi, qs])
                            ssp = ps_fil.tile([P, 512], f32, tag="fil",
                                              name=f"ss{qb}")
                            for mi in range(2):
                                x2c = x2p.tile([P, 512], bf16, tag="x2")
                                nc.vector.tensor_mul(
                                    x2c[:], xT[:, mi, qs], xT[:, mi, qs])
                                nc.tensor.matmul(
                                    ssp[0:1, :], ones_f[:, 0:1], x2c[:],
                                    start=(mi == 0), stop=(mi == 1))
                            ss_sb = x2p.tile([1, 512], bf16, tag="sssb")
                            nc.vector.tensor_copy(ss_sb[:], ssp[0:1, :])
                            nc.sync.dma_start(
                                ag_in[qb][2 * P:2 * P + 1, :], ss_sb[:])
                            nc.gpsimd.collective_compute(
                                "AllGather",
                                mybir.AluOpType.bypass,
                                replica_groups=GROUPS,
                                ins=[ag_in[qb].opt()],
                                outs=[ag_out[qb].opt()],
                            )

            if stop_after == "attn":
                nc.sync.dma_start(y_out_d[0:P, 0:512],
                                  xT[:, 0, :].bitcast(f32))
                return

            # ---- tail: gather, RMS scale, out-projection per query half ----
            with (
                tc.tile_pool(name="post", bufs=1) as post,
                tc.tile_pool(name="postr", bufs=2) as postr,
                tc.tile_pool(name="ps_yp", bufs=2, space="PSUM") as ps_yp,
                tc.tile_pool(name="ps_s", bufs=2, space="PSUM") as ps_s,
            ):
                for qb in range(2):
                    if qb == 1:
                        # keep the PE HAM clock-gate open while AG1 flies
                        warm = ps_yp.tile([P, 512], f32, tag="yp")
                        for _ in range(50):
                            nc.tensor.matmul(
                                warm[0:16, 0:16], ones_f[:, 0:16],
                                ones_f[:, 0:16], start=True, stop=True)
                    qs = slice(qb * 512, (qb + 1) * 512)
                    xall = post.tile([P, DC, 512], bf16, name=f"xall{qb}")
                    ss4 = post.tile([4, 512], bf16, name=f"ss4{qb}")
                    for r in range(G):
                        nc.sync.dma_start(
                            xall[:, r * 2:r * 2 + 2, :],
                            ag_out[qb][r * AGR:r * AGR + 2 * P, :].rearrange(
                                "(o p) n -> p o n", p=P))
                    for r in range(G):
                        nc.sync.dma_start(
                            ss4[r:r + 1, :],
                            ag_out[qb][r * AGR + 2 * P:r * AGR + 2 * P + 1, :])

                    s_ps = ps_s.tile([P, 512], f32, tag="sps")
                    nc.tensor.matmul(s_ps[:], ones4w[:], ss4[:],
                                     start=True, stop=True)
                    # s = 1/sqrt(ms+eps) = exp(-0.5*ln(ms+eps)): Ln and
                    # Exp share an ACT table set, Sqrt does not — this
                    # avoids a ~2.7us table switch on the critical tail.
                    s_b = post.tile([P, 512], f32, name=f"sb{qb}")
                    nc.scalar.activation(s_b[:], s_ps[:], AF.Ln,
                                         bias=eps_t[:],
                                         scale=1.0 / DIM)
                    nc.scalar.activation(s_b[:], s_b[:], AF.Exp,
                                         scale=-0.5)

                    for m in range(2):
                        yp = ps_yp.tile([P, 512], f32, tag="yp")
                        for j in range(DC):
                            nc.tensor.matmul(
                                yp[:],
                                wp_sb[:, j, m * P:(m + 1) * P],
                                xall[:, j, :],
                                start=(j == 0), stop=(j == DC - 1))
                        y_sb = postr.tile([P, 512], f32, tag="ysb")
                        nc.vector.tensor_mul(y_sb[:], yp[:], s_b[:])
                        nc.vector.tensor_scalar_add(y_sb[:], y_sb[:],
                                                    pb[:, m:m + 1])
                        nc.sync.dma_start(
                            y_out_d[m * P:(m + 1) * P, qs], y_sb[:])

    with tile.TileContext(nc) as tc:
        if loop_n:
            with tc.For_i(0, loop_n, 1):
                _trace(tc)
        else:
            _trace(tc)
    nc.compile()
    return nc


_CACHE = {}


def _get_nc():
    if "nc" not in _CACHE:
        _CACHE["nc"] = _build()
    return _CACHE["nc"]


def _shard_inputs(inputs):
    bf = ml_dtypes.bfloat16
    q = np.asarray(inputs["query"], np.float32)
    k = np.asarray(inputs["key"], np.float32)
    q1_w = np.asarray(inputs["q1_w"], np.float32)
    q2_w = np.asarray(inputs["q2_w"], np.float32)
    kv1_w = np.asarray(inputs["kv1_w"], np.float32)
    kv2_w = np.asarray(inputs["kv2_w"], np.float32)
    proj_w = np.asarray(inputs["proj_w"], np.float32)
    proj_b = np.asarray(inputs["proj_b"], np.float32)
    norm_w = np.asarray(inputs["norm_w"], np.float32)
    lam1 = np.asarray(inputs["lambda_1"], np.float32).reshape(H)
    lam2 = np.asarray(inputs["lambda_2"], np.float32).reshape(H)
    lam_full = lam1 - lam2 + LAMBDA_INIT

    # proj with norm folded in: y[o, q] = sum_c wpnw[c, o] x[c, q]
    wpnw = (proj_w * norm_w[None, :]).T  # [c, o]

    def c(x):
        return np.ascontiguousarray(x).astype(bf)

    in_maps = []
    for r in range(8):
        b, g = r // G, r % G
        rows = slice(g * 256, (g + 1) * 256)
        vrows = slice(DIM + g * 256, DIM + (g + 1) * 256)
        # AllGather chunk order: j = 2*rank + mi -> channels 256*rank+128*mi
        wp_perm = np.concatenate(
            [wpnw[rr * 256 + mi * 128: rr * 256 + (mi + 1) * 128, rows]
             for rr in range(G) for mi in range(2)], axis=0)
        in_maps.append({
            "qT": c(q[b].T),
            "kT": c(k[b].T),
            "wq1": c(q1_w[rows].T),
            "wq2": c(q2_w[rows].T),
            "wk1": c(kv1_w[rows].T),
            "wv1": c(kv1_w[vrows].T),
            "wk2": c(kv2_w[rows].T),
            "wp": c(wp_perm),
            "pb": np.ascontiguousarray(
                proj_b[rows].reshape(2, P).T).astype(np.float32),
            "lamn": np.ascontiguousarray(
                -lam_full[g * NH:(g + 1) * NH].reshape(1, NH)
            ).astype(np.float32),
        })
    return in_maps


def kernel(**inputs):
    from concourse.bass_utils import run_bass_kernel_spmd

    nc = _get_nc()
    in_maps = _shard_inputs(inputs)
    res = run_bass_kernel_spmd(nc, in_maps, core_ids=list(range(8)))
    out = np.empty((B, NQ, DIM), np.float32)
    for r in range(8):
        b, g = r // G, r % G
        out[b, :, g * 256:(g + 1) * 256] = res.results[r]["y_out"].T
    return out


# revision 7
# speedup vs baseline: 3.1045x; 2.7931x over previous
"""DIFF cross-attention kernel for 8 Trainium2 NeuronCores.

Sharding: tensor-parallel over heads x data-parallel over batch.
Core r handles batch b = r//4 and head group g = r%4 (4 of 16 heads).

The attention phase is ACT(exp)-bound (~137us of exp at 128 lanes), so
the whole schedule is arranged to hide everything else under it:
  - Prefix: PE-warmup dummies release the HAM clock gate during the DMA
    wait; one d-outer projection wave (q mi0 + k mi0 kc0/kc1, 8 PSUM
    banks) runs as the kT (SP queue) / weights+qT (ACT queue) streams
    land, emission order matched to arrival so the in-order PE FIFO
    never stalls early work behind late data.
  - Attention passes (hp=head-pair, qb=query-half, br=branch): per key
    tile ONE [128,1024] PSUM tile holds BOTH heads' scores — even head
    via PE row-tile (0,0), odd head via row-tile (64,0); the two K=64
    matmuls run concurrently on the PE array (HW-verified ~2x) and one
    exp covers both. The u-matmuls consuming slot k's exp are emitted in
    slot k+1 so the PE FIFO never waits on ACT.
  - v1e carries 64 ones-columns, so u rows 0:64 hold the softmax
    denominator replicated 64-wide: the branch combine needs no gpsimd
    partition-broadcast (reciprocal_approx_fast at partition base 0 —
    the custom DVE op breaks at non-zero base partitions).
  - Branch 1 is reduced to t1 = u1/l1 immediately (no PSUM evacuation
    copy, and the post-last-exp tail chain is short).
  - Remaining projections (v, k kc2/3, q/k mi=1) run as deadline-
    scheduled PE filler groups inside pass 1, so the first exp fires
    ~15us earlier than a serial prefix would allow.
  - One AllGather per (batch-group, query-half): qb0's collective and
    output projection hide under the qb1 attention pass / AG1 flight;
    dummy matmuls keep the PE clock-gate open across the AG1 wait.
  - RMS scale via s = exp(-0.5*ln(ms+eps)): Ln/Exp share an ACT table
    set (Sqrt does not), avoiding a table switch on the critical tail.

Tail math: RMSNorm's column scale commutes through the projection
(y = (W_nw x) * s + b), so norm_w is folded into the proj weights on
the host and the projection runs on raw gathered x with K=1024, each
core computing only its own 256 output rows -- no reduce needed.
Host returns out[b][t, c] = y_out[r][c', t].
"""

import numpy as np
import ml_dtypes

B = 2
NQ = 1024
NK = 2048
DIM = 1024
H = 16
HD = 64
NH = 4            # heads per core
G = 4             # cores per batch group
SCALE = 0.125
LAMBDA_INIT = 0.1
EPS = 1e-6
P = 128
DC = DIM // P     # 8 contraction chunks
KT = NK // P      # 16 key tiles
GROUPS = [[0, 1, 2, 3], [4, 5, 6, 7]]
AGR = 2 * P + 1   # rows per rank in the AllGather payload


def _build(stop_after="full", loop_n=0, pack=True):
    import concourse.bass as bass
    import concourse.tile as tile
    import concourse.mybir as mybir
    from concourse import bacc

    f32 = mybir.dt.float32
    bf16 = mybir.dt.bfloat16
    AF = mybir.ActivationFunctionType

    nc = bacc.Bacc("TRN2", target_bir_lowering=False, debug=False, num_devices=8)

    qT_d = nc.dram_tensor("qT", [DIM, NQ], bf16, kind="ExternalInput")
    kT_d = nc.dram_tensor("kT", [DIM, NK], bf16, kind="ExternalInput")
    wq1_d = nc.dram_tensor("wq1", [DIM, 256], bf16, kind="ExternalInput")
    wq2_d = nc.dram_tensor("wq2", [DIM, 256], bf16, kind="ExternalInput")
    wk1_d = nc.dram_tensor("wk1", [DIM, 256], bf16, kind="ExternalInput")
    wv1_d = nc.dram_tensor("wv1", [DIM, 256], bf16, kind="ExternalInput")
    wk2_d = nc.dram_tensor("wk2", [DIM, 256], bf16, kind="ExternalInput")
    # proj weights with norm_w folded in, rows pre-permuted to the
    # AllGather chunk order: chunk j=(r, mi) -> channels 256r+128mi..+128,
    # cols = this core's 256 output channels.
    wp_d = nc.dram_tensor("wp", [DIM, 256], bf16, kind="ExternalInput")
    pb_d = nc.dram_tensor("pb", [P, 2], f32, kind="ExternalInput")
    lamn_d = nc.dram_tensor("lamn", [1, NH], f32, kind="ExternalInput")
    y_out_d = nc.dram_tensor("y_out", [256, NQ], f32, kind="ExternalOutput")

    def _trace(tc):
        with (
            tc.tile_pool(name="res", bufs=1) as res,
            tc.tile_pool(name="dram", bufs=1, space="DRAM") as dram,
        ):
            # ---- resident tensors ----
            qT_sb = res.tile([P, DC, NQ], bf16)
            kt_sb = res.tile([P, DC, NK], bf16)
            wq1s = res.tile([P, DC, 256], bf16, name="wq1s")
            wq2s = res.tile([P, DC, 256], bf16, name="wq2s")
            wk1 = res.tile([P, DC, 256], bf16, name="wk1s")
            wk2 = res.tile([P, DC, 256], bf16, name="wk2s")
            wv1 = res.tile([P, DC, 256], bf16, name="wv1s")
            wp_sb = res.tile([P, DC, 256], bf16)
            pb = res.tile([P, 2], f32)
            lamn = res.tile([1, NH], f32)
            ones_f = res.tile([P, KT], bf16)
            ones4w = res.tile([4, P], bf16)
            lam64 = res.tile([64, NH], f32)
            eps_t = res.tile([P, 1], f32)

            q1T = res.tile([P, 2, NQ], bf16)
            q2T = res.tile([P, 2, NQ], bf16)
            k1T = res.tile([P, 2, NK], bf16)
            k2T = res.tile([P, 2, NK], bf16)
            v1e = res.tile([P, NH, KT, 128], bf16)
            xT = res.tile([P, 2, NQ], bf16)

            ag_in = [dram.tile([AGR, 512], bf16, name=f"agi{qb}")
                     for qb in range(2)]
            ag_out = [dram.tile([G * AGR, 512], bf16, name=f"ago{qb}")
                      for qb in range(2)]

            # ---- DMA stream, rough priority order. Config cost sits on
            # the issuing engine's sequencer (SP/DVE: HWDGE ~0.6us each,
            # transfers run parallel on the DMA queues; gpsimd: SWDGE ~1us
            # and engine-serialized) — so the critical q/k stream issues
            # from SP, weights from DVE, and tail-only loads from gpsimd.
            def rearr2(eng, t_, d_):
                for hh in range(2):
                    eng.dma_start(
                        t_[:, hh * 4:(hh + 1) * 4, :],
                        d_[hh * 512:(hh + 1) * 512, :].rearrange(
                            "(o p) n -> p o n", p=P))

            for d in range(DC):
                nc.sync.dma_start(kt_sb[:, d, :], kT_d[d * P:(d + 1) * P, :])
            for t_, d_ in ((wk1, wk1_d), (wk2, wk2_d), (wq1s, wq1_d),
                           (wq2s, wq2_d), (wv1, wv1_d)):
                rearr2(nc.scalar, t_, d_)
            for d in range(DC):
                nc.scalar.dma_start(qT_sb[:, d, :],
                                    qT_d[d * P:(d + 1) * P, :])
            rearr2(nc.gpsimd, wp_sb, wp_d)
            nc.gpsimd.dma_start(pb[:], pb_d[:])
            nc.gpsimd.dma_start(lamn[:], lamn_d[:])

            nc.vector.memset(ones_f[:], 1.0)
            nc.vector.memset(ones4w[:], 1.0)
            nc.vector.memset(eps_t[:], EPS)
            nc.gpsimd.memset(v1e[:, :, :, 0:64], 1.0)
            nc.gpsimd.partition_broadcast(lam64[:], lamn[:])

            def v_copy_out(pv, ktpair):
                """Copy a [P, 512] psum tile holding v-proj results for key
                tiles (2*ktpair, 2*ktpair+1) out to v1e."""
                for j in range(2):
                    kt = 2 * ktpair + j
                    for h in range(NH):
                        nc.vector.tensor_copy(
                            v1e[:, h, kt, 64:128],
                            pv[:, j * 256 + h * 64:j * 256 + h * 64 + 64])

            # ---- prefix: one d-outer wave (q-proj mi0 + k-proj mi0
            # kc0/kc1, 8 banks, gated on DMA arrival) ----
            with tc.tile_pool(name="ps_pre", bufs=8, space="PSUM") as ps_pre:
                psq = [ps_pre.tile([P, 512], f32, tag="acc", name=f"psq{i}")
                       for i in range(4)]
                psk = [ps_pre.tile([P, 512], f32, tag="acc", name=f"psk{i}")
                       for i in range(4)]
                # warm the PE HAM clock-gate during the DMA wait: dummy
                # matmuls into psq[0] (its real group later starts with
                # start=True, so junk contents are overwritten)
                for _ in range(48):
                    nc.tensor.matmul(psq[0][0:16, 0:16], ones_f[:, 0:16],
                                     ones_f[:, 0:16], start=True, stop=True)
                def k_d(d):
                    for i, ws in ((0, wk1), (1, wk2)):
                        for kc in range(2):
                            nc.tensor.matmul(
                                psk[i * 2 + kc][:], ws[:, d, 0:P],
                                kt_sb[:, d, kc * 512:(kc + 1) * 512],
                                start=(d == 0), stop=(d == DC - 1))

                def q_d(d):
                    for pj, ws in ((0, wq1s), (1, wq2s)):
                        for qc in range(2):
                            nc.tensor.matmul(
                                psq[pj * 2 + qc][:],
                                ws[:, d, 0:P],
                                qT_sb[:, d, qc * 512:(qc + 1) * 512],
                                start=(d == 0), stop=(d == DC - 1))

                # static emission order matched to DMA arrival (kT chunks
                # stream early on SP; qT lands later on the ACT queue) so
                # the in-order PE FIFO never parks an early k-chunk matmul
                # behind a q-chunk still in flight
                for step in ("k0", "k1", "k2", "k3", "q0", "k4", "q1",
                             "k5", "q2", "k6", "q3", "k7", "q4", "q5",
                             "q6", "q7"):
                    (k_d if step[0] == "k" else q_d)(int(step[1]))
                for pj, dst in ((0, q1T), (1, q2T)):
                    for qc in range(2):
                        nc.vector.tensor_copy(
                            dst[:, 0, qc * 512:(qc + 1) * 512],
                            psq[pj * 2 + qc][:])
                for i, dst in ((0, k1T), (1, k2T)):
                    for kc in range(2):
                        nc.vector.tensor_copy(
                            dst[:, 0, kc * 512:(kc + 1) * 512],
                            psk[i * 2 + kc][:])


            if stop_after == "prefix":
                nc.sync.dma_start(y_out_d[0:P, 0:512],
                                  q1T[:, 0, :].bitcast(f32))
                return
            if stop_after == "prefixk":
                nc.sync.dma_start(y_out_d[0:P, :],
                                  k1T[:, 0, :].bitcast(f32))
                nc.sync.dma_start(y_out_d[P:2 * P, 0:512].bitcast(bf16),
                                  v1e[:, 0, :, 0:64])
                nc.sync.dma_start(y_out_d[P:2 * P, 512:520].bitcast(bf16),
                                  v1e[:, 0, :, 64:65])
                return

            # ---- attention passes ----
            with (
                tc.tile_pool(name="ps_sc", bufs=2, space="PSUM") as ps_sc,
                tc.tile_pool(name="ps_u", bufs=2, space="PSUM") as ps_u,
                tc.tile_pool(name="ps_fil", bufs=2, space="PSUM") as ps_fil,
                tc.tile_pool(name="att", bufs=4) as att,
                tc.tile_pool(name="evac", bufs=4) as evac,
                tc.tile_pool(name="smal", bufs=4) as smal,
                tc.tile_pool(name="x2p", bufs=2) as x2p,
            ):
                def v_group(ktpair):
                    """v-proj for key tiles (2k, 2k+1) as a filler group."""
                    pv = ps_fil.tile([P, 512], f32, tag="fil",
                                     name=f"pv{ktpair}")
                    for j in range(2):
                        kt = 2 * ktpair + j
                        for d in range(DC):
                            nc.tensor.matmul(
                                pv[:, j * 256:(j + 1) * 256],
                                kt_sb[:, d, kt * P:(kt + 1) * P],
                                wv1[:, d, :],
                                start=(d == 0), stop=(d == DC - 1))
                    v_copy_out(pv[:], ktpair)

                def kc_group(ws, dst, kc):
                    """k-proj mi0 for key block kc as a filler group."""
                    pk = ps_fil.tile([P, 512], f32, tag="fil",
                                     name=f"pk{kc}")
                    for d in range(DC):
                        nc.tensor.matmul(
                            pk[:], ws[:, d, 0:P],
                            kt_sb[:, d, kc * 512:(kc + 1) * 512],
                            start=(d == 0), stop=(d == DC - 1))
                    nc.vector.tensor_copy(
                        dst[:, 0, kc * 512:(kc + 1) * 512], pk[:])

                # remaining projections emitted inside pass-1-br0, each
                # group just before the first slot that consumes it, so the
                # first exp fires ~15us earlier and ACT ramps while PE
                # drains these groups
                groups = [
                    (1, lambda: v_group(0)), (3, lambda: v_group(1)),
                    (5, lambda: v_group(2)), (7, lambda: v_group(3)),
                    (8, lambda: kc_group(wk1, k1T, 2)),
                    (9, lambda: v_group(4)),
                    (10, lambda: kc_group(wk2, k2T, 2)),
                    (11, lambda: v_group(5)),
                    (12, lambda: kc_group(wk1, k1T, 3)),
                    (13, lambda: v_group(6)),
                    (14, lambda: kc_group(wk2, k2T, 3)),
                    (15, lambda: v_group(7)),
                ]
                gq = list(groups)

                def emit_groups(slot):
                    while gq and gq[0][0] <= slot:
                        gq.pop(0)[1]()

                def filler_items():
                    """q-proj mi=1 then k-proj mi=1, one instruction per
                    item."""
                    for ws, dst in ((wq1s, q1T), (wq2s, q2T)):
                        for qc in range(2):
                            ft = ps_fil.tile([P, 512], f32, tag="fil",
                                             name=f"fq{qc}")
                            for d in range(DC):
                                yield lambda ft=ft, ws=ws, d=d, qc=qc: \
                                    nc.tensor.matmul(
                                        ft[:], ws[:, d, P:2 * P],
                                        qT_sb[:, d, qc * 512:(qc + 1) * 512],
                                        start=(d == 0), stop=(d == DC - 1))
                            yield lambda ft=ft, dst=dst, qc=qc: \
                                nc.vector.tensor_copy(
                                    dst[:, 1, qc * 512:(qc + 1) * 512], ft[:])
                    for ws, dst in ((wk1, k1T), (wk2, k2T)):
                        for kc in range(4):
                            ft = ps_fil.tile([P, 512], f32, tag="fil",
                                             name=f"fk{kc}")
                            for d in range(DC):
                                yield lambda ft=ft, ws=ws, d=d, kc=kc: \
                                    nc.tensor.matmul(
                                        ft[:], ws[:, d, P:2 * P],
                                        kt_sb[:, d, kc * 512:(kc + 1) * 512],
                                        start=(d == 0), stop=(d == DC - 1))
                            yield lambda ft=ft, dst=dst, kc=kc: \
                                nc.vector.tensor_copy(
                                    dst[:, 1, kc * 512:(kc + 1) * 512], ft[:])

                filler = filler_items()
                fill_done = False

                def emit_fill(k):
                    nonlocal fill_done
                    if fill_done:
                        return
                    for _ in range(k):
                        it = next(filler, None)
                        if it is None:
                            fill_done = True
                            return
                        it()

                for hp in range(2):
                    he, ho = 2 * hp, 2 * hp + 1
                    for qb in range(2):
                        qs = slice(qb * 512, (qb + 1) * 512)
                        u_prev = None
                        for br, (ktp, qtp) in enumerate(
                                ((k1T, q1T), (k2T, q2T))):
                            u_e = ps_u.tile([P, 512], f32, tag="u")
                            u_o = ps_u.tile([P, 512], f32, tag="u")
                            # software pipeline: the u-matmuls consuming
                            # slot k's exp are emitted during slot k+1, so
                            # the PE never sits in-FIFO behind the ACT exp
                            pend = None

                            def flush_u():
                                nonlocal pend
                                if pend is None:
                                    return
                                e_p, kt_p = pend
                                nc.tensor.matmul(
                                    u_e[:], v1e[:, he, kt_p, :],
                                    e_p[:, 0:512],
                                    start=(kt_p == 0), stop=(kt_p == KT - 1))
                                nc.tensor.matmul(
                                    u_o[:], v1e[:, ho, kt_p, :],
                                    e_p[:, 512:1024],
                                    start=(kt_p == 0), stop=(kt_p == KT - 1))
                                pend = None

                            for kt in range(KT):
                                if hp == 0 and qb == 0 and br == 0:
                                    emit_groups(kt)
                                sc = ps_sc.tile([P, 1024], f32, tag="sc")
                                nc.tensor.matmul(
                                    sc[:, 0:512],
                                    ktp[0:64, hp, kt * P:(kt + 1) * P],
                                    qtp[0:64, hp, qs],
                                    start=True, stop=True,
                                    tile_position=(0, 0) if pack else None)
                                nc.tensor.matmul(
                                    sc[:, 512:1024],
                                    ktp[64:128, hp, kt * P:(kt + 1) * P],
                                    qtp[64:128, hp, qs],
                                    start=True, stop=True,
                                    tile_position=(64, 0) if pack else None)
                                e_t = att.tile([P, 1024], bf16, tag="e")
                                nc.scalar.activation(e_t[:], sc[:],
                                                     AF.Exp, scale=SCALE)
                                flush_u()
                                pend = (e_t, kt)
                                if not gq:
                                    emit_fill(3)
                            flush_u()
                            if br == 0:
                                # reduce branch 1 to t1 = u1/l1 right here:
                                # frees the u banks as fast as an evacuation
                                # copy would, and takes the rr1/t1 ops off
                                # the post-last-exp critical tail
                                tps = []
                                for u1 in (u_e, u_o):
                                    rr1b = smal.tile([64, 512], f32,
                                                     tag="rrb")
                                    nc.vector.reciprocal_approx_fast(
                                        rr1b[:], u1[0:64, :])
                                    t1 = evac.tile([64, 512], f32, tag="u1")
                                    nc.vector.tensor_mul(
                                        t1[:], u1[64:128, :], rr1b[:])
                                    tps.append(t1)
                                u_prev = tuple(tps)
                            else:
                                for head, po, t1, u2 in (
                                        (he, 0, u_prev[0], u_e),
                                        (ho, 64, u_prev[1], u_o)):
                                    rr2b = smal.tile([64, 512], f32,
                                                     tag="rrb")
                                    nc.vector.reciprocal_approx_fast(
                                        rr2b[:], u2[0:64, :])
                                    nc.vector.tensor_scalar_mul(
                                        rr2b[:], rr2b[:],
                                        lam64[:, head:head + 1])
                                    t2 = smal.tile([64, 512], f32, tag="tt")
                                    nc.vector.tensor_mul(
                                        t2[:], u2[64:128, :], rr2b[:])
                                    nc.vector.tensor_add(
                                        xT[po:po + 64, hp, qs], t1[:], t2[:])
                        if hp == 1 and stop_after == "full":
                            # ss + AllGather for this query half
                            for mi in range(2):
                                nc.sync.dma_start(
                                    ag_in[qb][mi * P:(mi + 1) * P, :],
                                    xT[:, mi, qs])
                            ssp = ps_fil.tile([P, 512], f32, tag="fil",
                                              name=f"ss{qb}")
                            for mi in range(2):
                                x2c = x2p.tile([P, 512], bf16, tag="x2")
                                nc.vector.tensor_mul(
                                    x2c[:], xT[:, mi, qs], xT[:, mi, qs])
                                nc.tensor.matmul(
                                    ssp[0:1, :], ones_f[:, 0:1], x2c[:],
                                    start=(mi == 0), stop=(mi == 1))
                            ss_sb = x2p.tile([1, 512], bf16, tag="sssb")
                            nc.vector.tensor_copy(ss_sb[:], ssp[0:1, :])
                            nc.sync.dma_start(
                                ag_in[qb][2 * P:2 * P + 1, :], ss_sb[:])
                            nc.gpsimd.collective_compute(
                                "AllGather",
                                mybir.AluOpType.bypass,
                                replica_groups=GROUPS,
                                ins=[ag_in[qb].opt()],
                                outs=[ag_out[qb].opt()],
                            )

            if stop_after == "attn":
                nc.sync.dma_start(y_out_d[0:P, 0:512],
                                  xT[:, 0, :].bitcast(f32))
                return

            # ---- tail: gather, RMS scale, out-projection per query half ----
            with (
                tc.tile_pool(name="post", bufs=1) as post,
                tc.tile_pool(name="postr", bufs=2) as postr,
                tc.tile_pool(name="ps_yp", bufs=2, space="PSUM") as ps_yp,
                tc.tile_pool(name="ps_s", bufs=2, space="PSUM") as ps_s,
            ):
                for qb in range(2):
                    if qb == 1:
                        # keep the PE HAM clock-gate open while AG1 flies
                        warm = ps_yp.tile([P, 512], f32, tag="yp")
                        for _ in range(50):
                            nc.tensor.matmul(
                                warm[0:16, 0:16], ones_f[:, 0:16],
                                ones_f[:, 0:16], start=True, stop=True)
                    qs = slice(qb * 512, (qb + 1) * 512)
                    xall = post.tile([P, DC, 512], bf16, name=f"xall{qb}")
                    ss4 = post.tile([4, 512], bf16, name=f"ss4{qb}")
                    for r in range(G):
                        nc.sync.dma_start(
                            xall[:, r * 2:r * 2 + 2, :],
                            ag_out[qb][r * AGR:r * AGR + 2 * P, :].rearrange(
                                "(o p) n -> p o n", p=P))
                    for r in range(G):
                        nc.sync.dma_start(
                            ss4[r:r + 1, :],
                            ag_out[qb][r * AGR + 2 * P:r * AGR + 2 * P + 1, :])

                    s_ps = ps_s.tile([P, 512], f32, tag="sps")
                    nc.tensor.matmul(s_ps[:], ones4w[:], ss4[:],
                                     start=True, stop=True)
                    # s = 1/sqrt(ms+eps) = exp(-0.5*ln(ms+eps)): Ln and
                    # Exp share an ACT table set, Sqrt does not — this
                    # avoids a ~2.7us table switch on the critical tail.
                    s_b = post.tile([P, 512], f32, name=f"sb{qb}")
                    nc.scalar.activation(s_b[:], s_ps[:], AF.Ln,
                                         bias=eps_t[:],
                                         scale=1.0 / DIM)
                    nc.scalar.activation(s_b[:], s_b[:], AF.Exp,
                                         scale=-0.5)

                    for m in range(2):
                        yp = ps_yp.tile([P, 512], f32, tag="yp")
                        for j in range(DC):
                            nc.tensor.matmul(
                                yp[:],
                                wp_sb[:, j, m * P:(m + 1) * P],
                                xall[:, j, :],
                                start=(j == 0), stop=(j == DC - 1))
                        y_sb = postr.tile([P, 512], f32, tag="ysb")
                        nc.vector.tensor_mul(y_sb[:], yp[:], s_b[:])
                        nc.vector.tensor_scalar_add(y_sb[:], y_sb[:],
                                                    pb[:, m:m + 1])
                        nc.sync.dma_start(
                            y_out_d[m * P:(m + 1) * P, qs], y_sb[:])

    with tile.TileContext(nc) as tc:
        if loop_n:
            with tc.For_i(0, loop_n, 1):
                _trace(tc)
        else:
            _trace(tc)
    nc.compile()
    return nc


_CACHE = {}


def _get_nc():
    if "nc" not in _CACHE:
        _CACHE["nc"] = _build()
    return _CACHE["nc"]


def _shard_inputs(inputs):
    bf = ml_dtypes.bfloat16
    q = np.asarray(inputs["query"], np.float32)
    k = np.asarray(inputs["key"], np.float32)
    q1_w = np.asarray(inputs["q1_w"], np.float32)
    q2_w = np.asarray(inputs["q2_w"], np.float32)
    kv1_w = np.asarray(inputs["kv1_w"], np.float32)
    kv2_w = np.asarray(inputs["kv2_w"], np.float32)
    proj_w = np.asarray(inputs["proj_w"], np.float32)
    proj_b = np.asarray(inputs["proj_b"], np.float32)
    norm_w = np.asarray(inputs["norm_w"], np.float32)
    lam1 = np.asarray(inputs["lambda_1"], np.float32).reshape(H)
    lam2 = np.asarray(inputs["lambda_2"], np.float32).reshape(H)
    lam_full = lam1 - lam2 + LAMBDA_INIT

    # proj with norm folded in: y[o, q] = sum_c wpnw[c, o] x[c, q]
    wpnw = (proj_w * norm_w[None, :]).T  # [c, o]

    def c(x):
        return np.ascontiguousarray(x).astype(bf)

    in_maps = []
    for r in range(8):
        b, g = r // G, r % G
        rows = slice(g * 256, (g + 1) * 256)
        vrows = slice(DIM + g * 256, DIM + (g + 1) * 256)
        # AllGather chunk order: j = 2*rank + mi -> channels 256*rank+128*mi
        wp_perm = np.concatenate(
            [wpnw[rr * 256 + mi * 128: rr * 256 + (mi + 1) * 128, rows]
             for rr in range(G) for mi in range(2)], axis=0)
        in_maps.append({
            "qT": c(q[b].T),
            "kT": c(k[b].T),
            "wq1": c(q1_w[rows].T),
            "wq2": c(q2_w[rows].T),
            "wk1": c(kv1_w[rows].T),
            "wv1": c(kv1_w[vrows].T),
            "wk2": c(kv2_w[rows].T),
            "wp": c(wp_perm),
            "pb": np.ascontiguousarray(
                proj_b[rows].reshape(2, P).T).astype(np.float32),
            "lamn": np.ascontiguousarray(
                -lam_full[g * NH:(g + 1) * NH].reshape(1, NH)
            ).astype(np.float32),
        })
    return in_maps


def kernel(**inputs):
    from concourse.bass_utils import run_bass_kernel_spmd

    nc = _get_nc()
    in_maps = _shard_inputs(inputs)
    res = run_bass_kernel_spmd(nc, in_maps, core_ids=list(range(8)))
    out = np.empty((B, NQ, DIM), np.float32)
    for r in range(8):
        b, g = r // G, r % G
        out[b, :, g * 256:(g + 1) * 256] = res.results[r]["y_out"].T
    return out
